# revision 7
# baseline (speedup 1.0000x reference)
"""Trainium2 Bass kernel for a 2-layer GCN + TopK pooling + mean pool + linear head.

Reference computation (see problem):
  x = relu(gcn_conv(x, edge_index, W0, b0))
  x = relu(gcn_conv(x, edge_index, W1, b1))
  score = tanh((x @ pool_w) / ||pool_w||); top-K=250 of 500 per graph
  pooled = mean over kept nodes of (x * score); logits = pooled @ W_lin + b_lin
  out = log_softmax(logits)

Sharding: data-parallel over graphs. 104 padded graphs, 13 per core.
Each core aggregates (gather + one-hot matmul scatter) only the edges whose
*target* node lives in its 6500-node slab. Self-loops are appended to the
edge list so the GCN "+I" term and degree counting need no special cases.
Cross-core exchange: tiny AllGather of degrees, one AllGather of the
layer-2 gather table ([52000,64]).
"""

import os
import sys

for _p in ("/opt/trn_rl_repo", "/root/.axon_site/_ro/trn_rl_repo"):
    if os.path.isdir(_p) and _p not in sys.path:
        sys.path.insert(0, _p)

import numpy as np

import concourse.bacc as bacc
import concourse.bass as bass
import concourse.mybir as mybir
import concourse.tile as tile
from concourse.bass_utils import run_bass_kernel_spmd
from concourse.masks import make_identity

# ---- problem constants (hardcoded per contract) ----
N = 50000          # real nodes
E = 800000         # edges
G = 100            # graphs
NPG = 500          # nodes per graph
K = 250            # top-k per graph
D = 64
OUT = 10
NCORES = 8
BLK = 125          # nodes per aggregation block
GPC = 13           # graphs per core (padded to 104 graphs)
NPAD = NCORES * GPC * NPG      # 52000
NLOC = NPAD // NCORES          # 6500
LBLK = NLOC // BLK             # 52 local blocks per core
NBLK = NCORES * LBLK           # 416 global blocks
CH = 128                       # edges per chunk (matmul contraction size)

F32 = mybir.dt.float32
I32 = mybir.dt.int32

# gather-table / one-hot dtype: float32 (exact) or bfloat16 (2x DMA, 4x PE)
DT_TBL = mybir.dt.bfloat16 if os.environ.get("GNN_TBL_BF16", "1") == "1" else F32


def _preprocess(edge_index):
    """Bucket edges (plus self-loops) by target block; build per-core
    [128, TC] index/column arrays laid out chunk-major."""
    row = np.asarray(edge_index[0], dtype=np.int64)
    col = np.asarray(edge_index[1], dtype=np.int64)
    loops = np.arange(NPAD, dtype=np.int64)
    rows_all = np.concatenate([row, loops])
    cols_all = np.concatenate([col, loops])

    blk = (cols_all // BLK).astype(np.int64)          # global target block
    col_loc = (cols_all % BLK).astype(np.int64)

    order = np.argsort(blk, kind="stable")
    row_s = rows_all[order]
    colloc_s = col_loc[order]

    counts = np.bincount(blk, minlength=NBLK)
    cnts = counts.reshape(NCORES, LBLK)
    C_j = np.maximum(1, -(-cnts.max(axis=0) // CH))   # chunks per local block
    TC = int(C_j.sum())
    starts = np.zeros(LBLK, np.int64)
    starts[1:] = np.cumsum(C_j)[:-1]

    idx_row = np.zeros((NCORES, 128, TC), np.int32)
    col_lcl = np.full((NCORES, 128, TC), float(BLK), np.float32)  # pad -> no match
    bounds = np.concatenate([[0], np.cumsum(counts)])
    for k in range(NCORES):
        for j in range(LBLK):
            b = LBLK * k + j
            e0, e1 = int(bounds[b]), int(bounds[b + 1])
            n = e1 - e0
            if n == 0:
                continue
            o = int(starts[j])
            i = np.arange(n)
            idx_row[k, i % CH, o + i // CH] = row_s[e0:e1]
            col_lcl[k, i % CH, o + i // CH] = colloc_s[e0:e1]
    return idx_row, col_lcl, tuple(int(c) for c in C_j), TC


def _topk_mask(tc, out, in_, k_to_choose, min_val):
    """Mask of 1s where the top-k values per partition are (from
    concourse.kernels.top_k, inlined to fix a decorator/signature clash)."""
    nc = tc.nc
    KA = 8
    with tc.tile_pool(name="topk_sbuf", bufs=2) as sbuf_pool:
        tensor_on = in_
        for k_on in range(0, k_to_choose, KA):
            k_max = min(k_on + KA, k_to_choose)
            k_this = k_max - k_on
            mx = sbuf_pool.tile([in_.shape[0], KA], in_.dtype, tag="topk_mx")
            nc.vector.max(out=mx[:], in_=tensor_on)
            if k_this < KA:
                nc.vector.memset(mx[:, k_this:], min_val)
            nc.vector.match_replace(out=out, in_to_replace=mx[:],
                                    in_values=tensor_on, imm_value=min_val)
            tensor_on = out
        nc.vector.tensor_sub(out=out, in0=in_, in1=out)
        nc.vector.tensor_scalar_min(out, out, 1.0)


def _build_program(C_j, TC):
    nc = bacc.Bacc("TRN2", target_bir_lowering=False, debug=False,
                   num_devices=NCORES)

    xT = nc.dram_tensor("xT", [D, NPAD], F32, kind="ExternalInput").ap()
    W0 = nc.dram_tensor("W0", [D, D], F32, kind="ExternalInput").ap()
    W1 = nc.dram_tensor("W1", [D, D], F32, kind="ExternalInput").ap()
    Wl = nc.dram_tensor("Wl", [D, OUT], F32, kind="ExternalInput").ap()
    b0b = nc.dram_tensor("b0b", [128, D], F32, kind="ExternalInput").ap()
    b1b = nc.dram_tensor("b1b", [128, D], F32, kind="ExternalInput").ap()
    pwb = nc.dram_tensor("pwb", [128, D], F32, kind="ExternalInput").ap()
    blb = nc.dram_tensor("blb", [128, OUT], F32, kind="ExternalInput").ap()
    idxs = nc.dram_tensor("idxs", [128, TC], I32, kind="ExternalInput").ap()
    cols = nc.dram_tensor("cols", [128, TC], F32, kind="ExternalInput").ap()
    outp = nc.dram_tensor("out", [GPC, OUT], F32, kind="ExternalOutput").ap()

    g1t = nc.dram_tensor("g1t", [NPAD, D], DT_TBL, kind="Internal").ap()
    g2l = nc.dram_tensor("g2l", [NLOC, D], DT_TBL, kind="Internal").ap()
    g2t = nc.dram_tensor("g2t", [NPAD, D], DT_TBL, kind="Internal",
                         addr_space="Shared").ap()
    degl = nc.dram_tensor("degl", [NLOC], F32, kind="Internal").ap()
    degf = nc.dram_tensor("degf", [NPAD], F32, kind="Internal",
                          addr_space="Shared").ap()
    scd = nc.dram_tensor("scd", [NLOC], F32, kind="Internal").ap()
    wd = nc.dram_tensor("wd", [NLOC], F32, kind="Internal").ap()

    starts = [0] * LBLK
    for j in range(1, LBLK):
        starts[j] = starts[j - 1] + C_j[j - 1]
    Cmax = max(C_j)

    rg = [list(range(NCORES))]

    with tile.TileContext(nc) as tc:
        with (
            tc.tile_pool(name="const", bufs=1) as cpool,
            tc.tile_pool(name="slab", bufs=1) as slab,
            tc.tile_pool(name="oh", bufs=6) as ohpool,
            tc.tile_pool(name="gat", bufs=3) as gatpool,
            tc.tile_pool(name="xsl", bufs=3) as xpool,
            tc.tile_pool(name="tmp", bufs=6) as tpool,
            tc.tile_pool(name="ps_agg", bufs=3, space="PSUM") as ps_agg,
            tc.tile_pool(name="ps_mm", bufs=2, space="PSUM") as ps_mm,
            tc.tile_pool(name="ps_tr", bufs=2, space="PSUM") as ps_tr,
            tc.tile_pool(name="ps_acc", bufs=1, space="PSUM") as ps_acc,
        ):
            # ---- constants ----
            W0sb = cpool.tile([D, D], F32)
            W1sb = cpool.tile([D, D], F32)
            Wlsb = cpool.tile([D, OUT], F32)
            b0sb = cpool.tile([128, D], F32)
            b1sb = cpool.tile([128, D], F32)
            pwsb = cpool.tile([128, D], F32)
            blsb = cpool.tile([128, OUT], F32)
            nc.sync.dma_start(out=W0sb[:], in_=W0[:])
            nc.sync.dma_start(out=W1sb[:], in_=W1[:])
            nc.sync.dma_start(out=Wlsb[:], in_=Wl[:])
            nc.sync.dma_start(out=b0sb[:], in_=b0b[:])
            nc.sync.dma_start(out=b1sb[:], in_=b1b[:])
            nc.sync.dma_start(out=pwsb[:], in_=pwb[:])
            nc.sync.dma_start(out=blsb[:], in_=blb[:])

            idx_sb = slab.tile([128, TC], I32)
            col_sb = slab.tile([128, TC], F32)
            nc.sync.dma_start(out=idx_sb[:], in_=idxs[:])
            nc.sync.dma_start(out=col_sb[:], in_=cols[:])

            iota_i = cpool.tile([128, BLK], I32)
            iota_f = cpool.tile([128, BLK], F32)
            nc.gpsimd.iota(iota_i[:], pattern=[[1, BLK]], base=0,
                           channel_multiplier=0)
            nc.vector.tensor_copy(iota_f[:], iota_i[:])
            ones_t = cpool.tile([128, 1], DT_TBL)
            nc.vector.memset(ones_t[:], 1.0)
            ones_f = cpool.tile([128, 1], F32)
            nc.vector.memset(ones_f[:], 1.0)
            ident = cpool.tile([128, 128], F32)
            make_identity(nc, ident[:])

            deg_slab = slab.tile([BLK, LBLK], F32)
            disl = slab.tile([BLK, LBLK], F32)
            disf = slab.tile([BLK, NBLK], F32)
            out2_slab = slab.tile([BLK, LBLK * D], F32)
            sc_slab = slab.tile([BLK, LBLK], F32)

            def onehot(cc):
                oh = ohpool.tile([128, BLK], DT_TBL, tag="oh")
                nc.vector.tensor_tensor(
                    out=oh[:],
                    in0=col_sb[:, cc:cc + 1].to_broadcast([128, BLK]),
                    in1=iota_f[:],
                    op=mybir.AluOpType.is_equal,
                )
                return oh

            # ---- pass 0: degrees (count edges into each local node) ----
            for j in range(LBLK):
                ps = ps_agg.tile([BLK, 1], F32, tag="agg")
                for c in range(C_j[j]):
                    oh = onehot(starts[j] + c)
                    nc.tensor.matmul(ps[:], lhsT=oh[:], rhs=ones_t[:],
                                     start=(c == 0), stop=(c == C_j[j] - 1))
                nc.scalar.copy(deg_slab[:, j:j + 1], ps[:])

            # dis = 1/sqrt(deg) (deg >= 1 because of self-loops)
            nc.scalar.sqrt(disl[:], deg_slab[:])
            nc.vector.reciprocal(disl[:], disl[:])

            # AllGather degrees -> full dis table
            nc.sync.dma_start(
                out=degl.rearrange("(p j) -> p j", j=LBLK), in_=deg_slab[:])
            nc.gpsimd.collective_compute(
                "AllGather", mybir.AluOpType.bypass, replica_groups=rg,
                ins=[degl[:]], outs=[degf[:]])
            degf_slab = slab.tile([BLK, NBLK], F32)
            nc.sync.dma_start(
                out=degf_slab[:],
                in_=degf.rearrange("(k p j) -> p k j", k=NCORES, p=BLK, j=LBLK))
            nc.scalar.sqrt(disf[:], degf_slab[:])
            nc.vector.reciprocal(disf[:], disf[:])

            # ---- g1 table: g1 = dis * (x @ W0) for all nodes ----
            XS = 8  # blocks per xT slab
            for s in range(0, NBLK, XS):
                nb = min(XS, NBLK - s)
                xsl = xpool.tile([D, XS * BLK], F32, tag="xsl")
                nc.sync.dma_start(out=xsl[:, :nb * BLK],
                                  in_=xT[:, s * BLK:(s + nb) * BLK])
                for bi in range(nb):
                    b = s + bi
                    ph = ps_mm.tile([BLK, D], F32, tag="mm")
                    nc.tensor.matmul(ph[:], lhsT=xsl[:, bi * BLK:(bi + 1) * BLK],
                                     rhs=W0sb[:], start=True, stop=True)
                    g1b = tpool.tile([BLK, D], DT_TBL, tag="g1b")
                    nc.scalar.activation(g1b[:], ph[:],
                                         mybir.ActivationFunctionType.Copy,
                                         scale=disf[:, b:b + 1])
                    nc.sync.dma_start(out=g1t[b * BLK:(b + 1) * BLK, :], in_=g1b[:])

            # ---- layers 1 and 2 ----
            def layer(table, dst_is_g2, Wn, bn):
                for j in range(LBLK):
                    o, Cj = starts[j], C_j[j]
                    gat = gatpool.tile([128, Cmax * D], DT_TBL, tag="gat")
                    nc.gpsimd.indirect_dma_start(
                        out=gat[:, :Cj * D],
                        out_offset=None,
                        in_=table[:],
                        in_offset=bass.IndirectOffsetOnAxis(
                            ap=idx_sb[:, o:o + Cj], axis=0),
                    )
                    ps = ps_agg.tile([BLK, D], F32, tag="agg")
                    for c in range(Cj):
                        oh = onehot(o + c)
                        nc.tensor.matmul(ps[:], lhsT=oh[:],
                                         rhs=gat[:, c * D:(c + 1) * D],
                                         start=(c == 0), stop=(c == Cj - 1))
                    u = tpool.tile([BLK, D], F32, tag="u")
                    nc.scalar.activation(u[:], ps[:],
                                         mybir.ActivationFunctionType.Copy,
                                         scale=disl[:, j:j + 1])
                    nc.vector.tensor_add(u[:], u[:], bn[:BLK, :])
                    if dst_is_g2:
                        o1 = tpool.tile([BLK, D], F32, tag="o1")
                        nc.scalar.activation(o1[:], u[:],
                                             mybir.ActivationFunctionType.Relu)
                        # h2 = out1 @ W1 needs out1^T as lhsT
                        pt = ps_tr.tile([D, BLK], F32, tag="tr")
                        nc.tensor.transpose(pt[:], o1[:], ident[:BLK, :BLK])
                        t1 = tpool.tile([D, BLK], F32, tag="t1")
                        nc.vector.tensor_copy(t1[:], pt[:])
                        ph2 = ps_mm.tile([BLK, D], F32, tag="mm")
                        nc.tensor.matmul(ph2[:], lhsT=t1[:], rhs=Wn[:],
                                         start=True, stop=True)
                        g2b = tpool.tile([BLK, D], DT_TBL, tag="g2b")
                        nc.scalar.activation(g2b[:], ph2[:],
                                             mybir.ActivationFunctionType.Copy,
                                             scale=disl[:, j:j + 1])
                        nc.sync.dma_start(out=g2l[j * BLK:(j + 1) * BLK, :],
                                          in_=g2b[:])
                    else:
                        oc = out2_slab[:, j * D:(j + 1) * D]
                        nc.scalar.activation(oc, u[:],
                                             mybir.ActivationFunctionType.Relu)
                        sct = tpool.tile([BLK, D], F32, tag="sct")
                        nc.vector.tensor_mul(sct[:], oc, pwsb[:BLK, :])
                        nc.vector.tensor_reduce(
                            sc_slab[:, j:j + 1], sct[:],
                            axis=mybir.AxisListType.X, op=mybir.AluOpType.add)

            layer(g1t, True, W1sb, b0sb)
            nc.gpsimd.collective_compute(
                "AllGather", mybir.AluOpType.bypass, replica_groups=rg,
                ins=[g2l[:]], outs=[g2t[:]])
            layer(g2t, False, None, b1sb)

            # ---- pooling ----
            nc.scalar.activation(sc_slab[:], sc_slab[:],
                                 mybir.ActivationFunctionType.Tanh)
            # scores [125, 52] -> [52, 125] -> DRAM (node order) -> [13, 500]
            pst = ps_tr.tile([LBLK, BLK], F32, tag="tr")
            nc.tensor.transpose(pst[:], sc_slab[:], ident[:BLK, :BLK])
            scT = tpool.tile([LBLK, BLK], F32, tag="scT")
            nc.vector.tensor_copy(scT[:], pst[:])
            nc.sync.dma_start(out=scd.rearrange("(j p) -> j p", p=BLK), in_=scT[:])
            sc13 = slab.tile([GPC, NPG], F32)
            nc.sync.dma_start(out=sc13[:],
                              in_=scd.rearrange("(g n) -> g n", n=NPG))
            mask13 = slab.tile([GPC, NPG], F32)
            _topk_mask(tc, mask13[:], sc13[:], K, min_val=-4.0)
            w13 = slab.tile([GPC, NPG], F32)
            nc.vector.tensor_mul(w13[:], mask13[:], sc13[:])
            nc.sync.dma_start(out=wd.rearrange("(g n) -> g n", n=NPG), in_=w13[:])
            wT = tpool.tile([LBLK, BLK], F32, tag="wT")
            nc.sync.dma_start(out=wT[:], in_=wd.rearrange("(j p) -> j p", p=BLK))
            psw = ps_tr.tile([BLK, LBLK], F32, tag="tr")
            nc.tensor.transpose(psw[:], wT[:], ident[:LBLK, :LBLK])
            w_slab = slab.tile([BLK, LBLK], F32)
            nc.vector.tensor_copy(w_slab[:], psw[:])

            # pooledT[d, gr] = sum_n w[n] * out2[n, d]
            psp = ps_acc.tile([D, GPC], F32, tag="acc")
            for j in range(LBLK):
                wx = tpool.tile([BLK, D], F32, tag="wx")
                nc.vector.tensor_mul(wx[:], out2_slab[:, j * D:(j + 1) * D],
                                     w_slab[:, j:j + 1].to_broadcast([BLK, D]))
                gr = j // 4
                nc.tensor.matmul(psp[:, gr:gr + 1], lhsT=wx[:], rhs=ones_f[:BLK, :],
                                 start=(j % 4 == 0), stop=(j % 4 == 3))
            pooledT = tpool.tile([D, GPC], F32, tag="pooledT")
            nc.scalar.activation(pooledT[:], psp[:],
                                 mybir.ActivationFunctionType.Copy,
                                 scale=1.0 / K)

            psl = ps_mm.tile([GPC, OUT], F32, tag="mm")
            nc.tensor.matmul(psl[:], lhsT=pooledT[:], rhs=Wlsb[:],
                             start=True, stop=True)
            lg = tpool.tile([GPC, OUT], F32, tag="lg")
            nc.vector.tensor_add(lg[:], psl[:], blsb[:GPC, :])
            mx = tpool.tile([GPC, 1], F32, tag="mx")
            nc.vector.tensor_reduce(mx[:], lg[:], axis=mybir.AxisListType.X,
                                    op=mybir.AluOpType.max)
            nmx = tpool.tile([GPC, 1], F32, tag="nmx")
            nc.vector.tensor_scalar_mul(nmx[:], mx[:], -1.0)
            ex = tpool.tile([GPC, OUT], F32, tag="ex")
            nc.scalar.activation(ex[:], lg[:], mybir.ActivationFunctionType.Exp,
                                 bias=nmx[:])
            se = tpool.tile([GPC, 1], F32, tag="se")
            nc.vector.tensor_reduce(se[:], ex[:], axis=mybir.AxisListType.X,
                                    op=mybir.AluOpType.add)
            ls = tpool.tile([GPC, 1], F32, tag="ls")
            nc.scalar.activation(ls[:], se[:], mybir.ActivationFunctionType.Ln)
            m2 = tpool.tile([GPC, 1], F32, tag="m2")
            nc.vector.tensor_add(m2[:], mx[:], ls[:])
            res = tpool.tile([GPC, OUT], F32, tag="res")
            nc.vector.tensor_sub(res[:], lg[:], m2[:].to_broadcast([GPC, OUT]))
            nc.sync.dma_start(out=outp[:], in_=res[:])

    nc.compile()
    return nc


_CACHE = {}


def _get_program(C_j, TC):
    key = (C_j, TC, str(DT_TBL))
    if key not in _CACHE:
        _CACHE[key] = _build_program(C_j, TC)
    return _CACHE[key]


def make_in_maps(x, edge_index, W0, b0, W1, b1, pool_w, W_lin, b_lin):
    x = np.asarray(x, np.float32)
    x_pad = np.zeros((NPAD, D), np.float32)
    x_pad[:N] = x
    xT = np.ascontiguousarray(x_pad.T)

    idx_row, col_lcl, C_j, TC = _preprocess(np.asarray(edge_index))

    pw = np.asarray(pool_w, np.float32)
    pwn = (pw / np.linalg.norm(pw)).astype(np.float32)

    def bc(v, n):
        return np.ascontiguousarray(
            np.broadcast_to(np.asarray(v, np.float32), (128, n)))

    common = {
        "xT": xT,
        "W0": np.asarray(W0, np.float32),
        "W1": np.asarray(W1, np.float32),
        "Wl": np.asarray(W_lin, np.float32),
        "b0b": bc(b0, D),
        "b1b": bc(b1, D),
        "pwb": bc(pwn, D),
        "blb": bc(b_lin, OUT),
    }
    in_maps = [
        dict(common, idxs=np.ascontiguousarray(idx_row[k]),
             cols=np.ascontiguousarray(col_lcl[k]))
        for k in range(NCORES)
    ]
    return in_maps, C_j, TC


def kernel(x, edge_index, batch, W0, b0, W1, b1, pool_w, W_lin, b_lin):
    in_maps, C_j, TC = make_in_maps(x, edge_index, W0, b0, W1, b1,
                                    pool_w, W_lin, b_lin)
    nc = _get_program(C_j, TC)
    res = run_bass_kernel_spmd(nc, in_maps, core_ids=list(range(NCORES)))
    out = np.concatenate([res.results[k]["out"] for k in range(NCORES)], axis=0)
    return np.ascontiguousarray(out[:G])


# revision 12
# speedup vs baseline: 69.7269x; 69.7269x over previous
"""Trainium2 Bass kernel for a 2-layer GCN + TopK pooling + mean pool + linear head.

Reference computation (see problem):
  x = relu(gcn_conv(x, edge_index, W0, b0))
  x = relu(gcn_conv(x, edge_index, W1, b1))
  score = tanh((x @ pool_w) / ||pool_w||); top-K=250 of 500 per graph
  pooled = mean over kept nodes of (x * score); logits = pooled @ W_lin + b_lin
  out = log_softmax(logits)

Sharding: data-parallel over graphs. 104 padded graphs, 13 per core.
Each core aggregates (gather + one-hot matmul scatter) only the edges whose
*target* node lives in its 6500-node slab. Self-loops are appended to the
edge list so the GCN "+I" term and degree counting need no special cases.
Cross-core exchange: tiny AllGather of degrees, one AllGather of the
layer-2 gather table ([52000,64]).
"""

import os
import sys

for _p in ("/opt/trn_rl_repo", "/root/.axon_site/_ro/trn_rl_repo"):
    if os.path.isdir(_p) and _p not in sys.path:
        sys.path.insert(0, _p)

import numpy as np

import concourse.bacc as bacc
import concourse.bass as bass
import concourse.mybir as mybir
import concourse.tile as tile
from concourse.bass_utils import run_bass_kernel_spmd
from concourse.masks import make_identity

# ---- problem constants (hardcoded per contract) ----
N = 50000          # real nodes
E = 800000         # edges
G = 100            # graphs
NPG = 500          # nodes per graph
K = 250            # top-k per graph
D = 64
OUT = 10
NCORES = 8
BLK = 125          # nodes per aggregation block
GPC = 13           # graphs per core (padded to 104 graphs)
NPAD = NCORES * GPC * NPG      # 52000
NLOC = NPAD // NCORES          # 6500
LBLK = NLOC // BLK             # 52 local blocks per core
NBLK = NCORES * LBLK           # 416 global blocks
CH = 128                       # edges per chunk (matmul contraction size)

F32 = mybir.dt.float32
I32 = mybir.dt.int32

# gather-table / one-hot dtype: float32 (exact) or bfloat16 (2x DMA, 4x PE)
DT_TBL = mybir.dt.bfloat16 if os.environ.get("GNN_TBL_BF16", "1") == "1" else F32


def _preprocess(edge_index):
    """Bucket edges (plus self-loops) by target block; build per-core
    [128, TC] index/column arrays laid out chunk-major."""
    row = np.asarray(edge_index[0], dtype=np.int64)
    col = np.asarray(edge_index[1], dtype=np.int64)
    loops = np.arange(NPAD, dtype=np.int64)
    rows_all = np.concatenate([row, loops])
    cols_all = np.concatenate([col, loops])

    blk = (cols_all // BLK).astype(np.int64)          # global target block
    col_loc = (cols_all % BLK).astype(np.int64)

    order = np.argsort(blk, kind="stable")
    row_s = rows_all[order]
    colloc_s = col_loc[order]

    counts = np.bincount(blk, minlength=NBLK)
    cnts = counts.reshape(NCORES, LBLK)
    C_j = np.maximum(1, -(-cnts.max(axis=0) // CH))   # chunks per local block
    TC = int(C_j.sum())
    starts = np.zeros(LBLK, np.int64)
    starts[1:] = np.cumsum(C_j)[:-1]

    idx_row = np.zeros((NCORES, 128, TC), np.int32)
    col_lcl = np.full((NCORES, 128, TC), float(BLK), np.float32)  # pad -> no match
    bounds = np.concatenate([[0], np.cumsum(counts)])
    blk_sorted = blk[order]
    rank = np.arange(len(blk_sorted)) - bounds[blk_sorted]  # rank within block
    kk = blk_sorted // LBLK
    jj = blk_sorted % LBLK
    pp = rank % CH
    cc = starts[jj] + rank // CH
    idx_row[kk, pp, cc] = row_s
    col_lcl[kk, pp, cc] = colloc_s
    return idx_row, col_lcl, tuple(int(c) for c in C_j), TC


def _topk_mask(tc, out, in_, k_to_choose, min_val):
    """Mask of 1s where the top-k values per partition are (from
    concourse.kernels.top_k, inlined to fix a decorator/signature clash)."""
    nc = tc.nc
    KA = 8
    with tc.tile_pool(name="topk_sbuf", bufs=2) as sbuf_pool:
        tensor_on = in_
        for k_on in range(0, k_to_choose, KA):
            k_max = min(k_on + KA, k_to_choose)
            k_this = k_max - k_on
            mx = sbuf_pool.tile([in_.shape[0], KA], in_.dtype, tag="topk_mx")
            nc.vector.max(out=mx[:], in_=tensor_on)
            if k_this < KA:
                nc.vector.memset(mx[:, k_this:], min_val)
            nc.vector.match_replace(out=out, in_to_replace=mx[:],
                                    in_values=tensor_on, imm_value=min_val)
            tensor_on = out
        nc.vector.tensor_sub(out=out, in0=in_, in1=out)
        nc.vector.tensor_scalar_min(out, out, 1.0)


def _build_program(C_j, TC):
    nc = bacc.Bacc("TRN2", target_bir_lowering=False, debug=False,
                   num_devices=NCORES)

    xT = nc.dram_tensor("xT", [D, NPAD], F32, kind="ExternalInput").ap()
    W0 = nc.dram_tensor("W0", [D, D], F32, kind="ExternalInput").ap()
    W1 = nc.dram_tensor("W1", [D, D], F32, kind="ExternalInput").ap()
    Wl = nc.dram_tensor("Wl", [D, OUT], F32, kind="ExternalInput").ap()
    b0b = nc.dram_tensor("b0b", [128, D], F32, kind="ExternalInput").ap()
    b1b = nc.dram_tensor("b1b", [128, D], F32, kind="ExternalInput").ap()
    pwb = nc.dram_tensor("pwb", [128, D], F32, kind="ExternalInput").ap()
    blb = nc.dram_tensor("blb", [128, OUT], F32, kind="ExternalInput").ap()
    idxs = nc.dram_tensor("idxs", [128, TC], I32, kind="ExternalInput").ap()
    cols = nc.dram_tensor("cols", [128, TC], F32, kind="ExternalInput").ap()
    outp = nc.dram_tensor("out", [GPC, OUT], F32, kind="ExternalOutput").ap()

    g1t = nc.dram_tensor("g1t", [NPAD, D], DT_TBL, kind="Internal").ap()
    g2l = nc.dram_tensor("g2l", [NLOC, D], DT_TBL, kind="Internal").ap()
    g2t = nc.dram_tensor("g2t", [NPAD, D], DT_TBL, kind="Internal",
                         addr_space="Shared").ap()
    degl = nc.dram_tensor("degl", [NLOC], F32, kind="Internal").ap()
    degf = nc.dram_tensor("degf", [NPAD], F32, kind="Internal",
                          addr_space="Shared").ap()
    scd = nc.dram_tensor("scd", [NLOC], F32, kind="Internal").ap()
    wd = nc.dram_tensor("wd", [NLOC], F32, kind="Internal").ap()

    starts = [0] * LBLK
    for j in range(1, LBLK):
        starts[j] = starts[j - 1] + C_j[j - 1]
    Cmax = max(C_j)

    rg = [list(range(NCORES))]

    with tile.TileContext(nc) as tc:
        with (
            tc.tile_pool(name="const", bufs=1) as cpool,
            tc.tile_pool(name="slab", bufs=1) as slab,
            tc.tile_pool(name="oh", bufs=6) as ohpool,
            tc.tile_pool(name="gat", bufs=3) as gatpool,
            tc.tile_pool(name="xsl", bufs=3) as xpool,
            tc.tile_pool(name="tmp", bufs=6) as tpool,
            tc.tile_pool(name="ps_agg", bufs=3, space="PSUM") as ps_agg,
            tc.tile_pool(name="ps_mm", bufs=2, space="PSUM") as ps_mm,
            tc.tile_pool(name="ps_tr", bufs=2, space="PSUM") as ps_tr,
            tc.tile_pool(name="ps_acc", bufs=1, space="PSUM") as ps_acc,
        ):
            # ---- constants ----
            W0sb = cpool.tile([D, D], F32)
            W1sb = cpool.tile([D, D], F32)
            Wlsb = cpool.tile([D, OUT], F32)
            b0sb = cpool.tile([128, D], F32)
            b1sb = cpool.tile([128, D], F32)
            pwsb = cpool.tile([128, D], F32)
            blsb = cpool.tile([128, OUT], F32)
            nc.sync.dma_start(out=W0sb[:], in_=W0[:])
            nc.sync.dma_start(out=W1sb[:], in_=W1[:])
            nc.sync.dma_start(out=Wlsb[:], in_=Wl[:])
            nc.sync.dma_start(out=b0sb[:], in_=b0b[:])
            nc.sync.dma_start(out=b1sb[:], in_=b1b[:])
            nc.sync.dma_start(out=pwsb[:], in_=pwb[:])
            nc.sync.dma_start(out=blsb[:], in_=blb[:])

            idx_sb = slab.tile([128, TC], I32)
            col_sb = slab.tile([128, TC], F32)
            nc.sync.dma_start(out=idx_sb[:], in_=idxs[:])
            nc.sync.dma_start(out=col_sb[:], in_=cols[:])

            iota_i = cpool.tile([128, BLK], I32)
            iota_f = cpool.tile([128, BLK], F32)
            nc.gpsimd.iota(iota_i[:], pattern=[[1, BLK]], base=0,
                           channel_multiplier=0)
            nc.vector.tensor_copy(iota_f[:], iota_i[:])
            ones_t = cpool.tile([128, 1], DT_TBL)
            nc.vector.memset(ones_t[:], 1.0)
            ones_f = cpool.tile([128, 1], F32)
            nc.vector.memset(ones_f[:], 1.0)
            ident = cpool.tile([128, 128], F32)
            make_identity(nc, ident[:])

            deg_slab = slab.tile([BLK, LBLK], F32)
            disl = slab.tile([BLK, LBLK], F32)
            disf = slab.tile([BLK, NBLK], F32)
            out2_slab = slab.tile([BLK, LBLK * D], F32)
            sc_slab = slab.tile([BLK, LBLK], F32)

            def onehot(cc):
                oh = ohpool.tile([128, BLK], DT_TBL, tag="oh")
                nc.vector.tensor_tensor(
                    out=oh[:],
                    in0=col_sb[:, cc:cc + 1].to_broadcast([128, BLK]),
                    in1=iota_f[:],
                    op=mybir.AluOpType.is_equal,
                )
                return oh

            # ---- pass 0: degrees (count edges into each local node) ----
            for j in range(LBLK):
                ps = ps_agg.tile([BLK, 1], F32, tag="agg")
                for c in range(C_j[j]):
                    oh = onehot(starts[j] + c)
                    nc.tensor.matmul(ps[:], lhsT=oh[:], rhs=ones_t[:],
                                     start=(c == 0), stop=(c == C_j[j] - 1))
                nc.scalar.copy(deg_slab[:, j:j + 1], ps[:])

            # dis = 1/sqrt(deg) (deg >= 1 because of self-loops)
            nc.scalar.sqrt(disl[:], deg_slab[:])
            nc.vector.reciprocal(disl[:], disl[:])

            # AllGather degrees -> full dis table
            nc.sync.dma_start(
                out=degl.rearrange("(p j) -> p j", j=LBLK), in_=deg_slab[:])
            nc.gpsimd.collective_compute(
                "AllGather", mybir.AluOpType.bypass, replica_groups=rg,
                ins=[degl[:]], outs=[degf[:]])
            degf_slab = slab.tile([BLK, NBLK], F32)
            nc.sync.dma_start(
                out=degf_slab[:],
                in_=degf.rearrange("(k p j) -> p k j", k=NCORES, p=BLK, j=LBLK))
            nc.scalar.sqrt(disf[:], degf_slab[:])
            nc.vector.reciprocal(disf[:], disf[:])

            # ---- g1 table: g1 = dis * (x @ W0) for all nodes ----
            XS = 8  # blocks per xT slab
            for s in range(0, NBLK, XS):
                nb = min(XS, NBLK - s)
                xsl = xpool.tile([D, XS * BLK], F32, tag="xsl")
                nc.sync.dma_start(out=xsl[:, :nb * BLK],
                                  in_=xT[:, s * BLK:(s + nb) * BLK])
                for bi in range(nb):
                    b = s + bi
                    ph = ps_mm.tile([BLK, D], F32, tag="mm")
                    nc.tensor.matmul(ph[:], lhsT=xsl[:, bi * BLK:(bi + 1) * BLK],
                                     rhs=W0sb[:], start=True, stop=True)
                    g1b = tpool.tile([BLK, D], DT_TBL, tag="g1b")
                    nc.scalar.activation(g1b[:], ph[:],
                                         mybir.ActivationFunctionType.Copy,
                                         scale=disf[:, b:b + 1])
                    nc.sync.dma_start(out=g1t[b * BLK:(b + 1) * BLK, :], in_=g1b[:])

            # ---- layers 1 and 2 ----
            def layer(table, dst_is_g2, Wn, bn):
                for j in range(LBLK):
                    o, Cj = starts[j], C_j[j]
                    gat = gatpool.tile([128, Cmax * D], DT_TBL, tag="gat")
                    nc.gpsimd.indirect_dma_start(
                        out=gat[:, :Cj * D],
                        out_offset=None,
                        in_=table[:],
                        in_offset=bass.IndirectOffsetOnAxis(
                            ap=idx_sb[:, o:o + Cj], axis=0),
                    )
                    ps = ps_agg.tile([BLK, D], F32, tag="agg")
                    for c in range(Cj):
                        oh = onehot(o + c)
                        nc.tensor.matmul(ps[:], lhsT=oh[:],
                                         rhs=gat[:, c * D:(c + 1) * D],
                                         start=(c == 0), stop=(c == Cj - 1))
                    u = tpool.tile([BLK, D], F32, tag="u")
                    nc.scalar.activation(u[:], ps[:],
                                         mybir.ActivationFunctionType.Copy,
                                         scale=disl[:, j:j + 1])
                    nc.vector.tensor_add(u[:], u[:], bn[:BLK, :])
                    if dst_is_g2:
                        o1 = tpool.tile([BLK, D], F32, tag="o1")
                        nc.scalar.activation(o1[:], u[:],
                                             mybir.ActivationFunctionType.Relu)
                        # h2 = out1 @ W1 needs out1^T as lhsT
                        pt = ps_tr.tile([D, BLK], F32, tag="tr")
                        nc.tensor.transpose(pt[:], o1[:], ident[:BLK, :BLK])
                        t1 = tpool.tile([D, BLK], F32, tag="t1")
                        nc.vector.tensor_copy(t1[:], pt[:])
                        ph2 = ps_mm.tile([BLK, D], F32, tag="mm")
                        nc.tensor.matmul(ph2[:], lhsT=t1[:], rhs=Wn[:],
                                         start=True, stop=True)
                        g2b = tpool.tile([BLK, D], DT_TBL, tag="g2b")
                        nc.scalar.activation(g2b[:], ph2[:],
                                             mybir.ActivationFunctionType.Copy,
                                             scale=disl[:, j:j + 1])
                        nc.sync.dma_start(out=g2l[j * BLK:(j + 1) * BLK, :],
                                          in_=g2b[:])
                    else:
                        oc = out2_slab[:, j * D:(j + 1) * D]
                        nc.scalar.activation(oc, u[:],
                                             mybir.ActivationFunctionType.Relu)
                        sct = tpool.tile([BLK, D], F32, tag="sct")
                        nc.vector.tensor_mul(sct[:], oc, pwsb[:BLK, :])
                        nc.vector.tensor_reduce(
                            sc_slab[:, j:j + 1], sct[:],
                            axis=mybir.AxisListType.X, op=mybir.AluOpType.add)

            layer(g1t, True, W1sb, b0sb)
            nc.gpsimd.collective_compute(
                "AllGather", mybir.AluOpType.bypass, replica_groups=rg,
                ins=[g2l[:]], outs=[g2t[:]])
            layer(g2t, False, None, b1sb)

            # ---- pooling ----
            nc.scalar.activation(sc_slab[:], sc_slab[:],
                                 mybir.ActivationFunctionType.Tanh)
            # scores [125, 52] -> [52, 125] -> DRAM (node order) -> [13, 500]
            pst = ps_tr.tile([LBLK, BLK], F32, tag="tr")
            nc.tensor.transpose(pst[:], sc_slab[:], ident[:BLK, :BLK])
            scT = tpool.tile([LBLK, BLK], F32, tag="scT")
            nc.vector.tensor_copy(scT[:], pst[:])
            nc.sync.dma_start(out=scd.rearrange("(j p) -> j p", p=BLK), in_=scT[:])
            sc13 = slab.tile([GPC, NPG], F32)
            nc.sync.dma_start(out=sc13[:],
                              in_=scd.rearrange("(g n) -> g n", n=NPG))
            mask13 = slab.tile([GPC, NPG], F32)
            _topk_mask(tc, mask13[:], sc13[:], K, min_val=-4.0)
            w13 = slab.tile([GPC, NPG], F32)
            nc.vector.tensor_mul(w13[:], mask13[:], sc13[:])
            nc.sync.dma_start(out=wd.rearrange("(g n) -> g n", n=NPG), in_=w13[:])
            wT = tpool.tile([LBLK, BLK], F32, tag="wT")
            nc.sync.dma_start(out=wT[:], in_=wd.rearrange("(j p) -> j p", p=BLK))
            psw = ps_tr.tile([BLK, LBLK], F32, tag="tr")
            nc.tensor.transpose(psw[:], wT[:], ident[:LBLK, :LBLK])
            w_slab = slab.tile([BLK, LBLK], F32)
            nc.vector.tensor_copy(w_slab[:], psw[:])

            # pooledT[d, gr] = sum_n w[n] * out2[n, d]
            psp = ps_acc.tile([D, GPC], F32, tag="acc")
            for j in range(LBLK):
                wx = tpool.tile([BLK, D], F32, tag="wx")
                nc.vector.tensor_mul(wx[:], out2_slab[:, j * D:(j + 1) * D],
                                     w_slab[:, j:j + 1].to_broadcast([BLK, D]))
                gr = j // 4
                nc.tensor.matmul(psp[:, gr:gr + 1], lhsT=wx[:], rhs=ones_f[:BLK, :],
                                 start=(j % 4 == 0), stop=(j % 4 == 3))
            pooledT = tpool.tile([D, GPC], F32, tag="pooledT")
            nc.scalar.activation(pooledT[:], psp[:],
                                 mybir.ActivationFunctionType.Copy,
                                 scale=1.0 / K)

            psl = ps_mm.tile([GPC, OUT], F32, tag="mm")
            nc.tensor.matmul(psl[:], lhsT=pooledT[:], rhs=Wlsb[:],
                             start=True, stop=True)
            lg = tpool.tile([GPC, OUT], F32, tag="lg")
            nc.vector.tensor_add(lg[:], psl[:], blsb[:GPC, :])
            mx = tpool.tile([GPC, 1], F32, tag="mx")
            nc.vector.tensor_reduce(mx[:], lg[:], axis=mybir.AxisListType.X,
                                    op=mybir.AluOpType.max)
            nmx = tpool.tile([GPC, 1], F32, tag="nmx")
            nc.vector.tensor_scalar_mul(nmx[:], mx[:], -1.0)
            ex = tpool.tile([GPC, OUT], F32, tag="ex")
            nc.scalar.activation(ex[:], lg[:], mybir.ActivationFunctionType.Exp,
                                 bias=nmx[:])
            se = tpool.tile([GPC, 1], F32, tag="se")
            nc.vector.tensor_reduce(se[:], ex[:], axis=mybir.AxisListType.X,
                                    op=mybir.AluOpType.add)
            ls = tpool.tile([GPC, 1], F32, tag="ls")
            nc.scalar.activation(ls[:], se[:], mybir.ActivationFunctionType.Ln)
            m2 = tpool.tile([GPC, 1], F32, tag="m2")
            nc.vector.tensor_add(m2[:], mx[:], ls[:])
            res = tpool.tile([GPC, OUT], F32, tag="res")
            nc.vector.tensor_sub(res[:], lg[:], m2[:].to_broadcast([GPC, OUT]))
            nc.sync.dma_start(out=outp[:], in_=res[:])

    nc.compile()
    return nc


class _Runner:
    """Caches the jitted PJRT executable for repeated invocations.

    Mirrors bass2jax.run_bass_via_pjrt's multi-core path, but keeps the
    jitted shard_map callable (and optionally device-resident inputs) so
    repeat calls skip retracing/relowering and input re-upload.
    """

    def __init__(self, nc):
        import jax
        from jax.sharding import Mesh, PartitionSpec, NamedSharding
        from jax.experimental.shard_map import shard_map
        from concourse import bass2jax

        bass2jax.install_neuronx_cc_hook()
        self.jax = jax
        self.nc = nc
        partition_name = (nc.partition_id_tensor.name
                          if nc.partition_id_tensor else None)
        in_names, out_names, out_avals, zero_outs = [], [], [], []
        for alloc in nc.m.functions[0].allocations:
            if not isinstance(alloc, mybir.MemoryLocationSet):
                continue
            name = alloc.memorylocations[0].name
            if alloc.kind == "ExternalInput":
                if name != partition_name:
                    in_names.append(name)
            elif alloc.kind == "ExternalOutput":
                shape = tuple(alloc.tensor_shape)
                dtype = mybir.dt.np(alloc.dtype)
                out_names.append(name)
                out_avals.append(jax.core.ShapedArray(shape, dtype))
                zero_outs.append(np.zeros(shape, dtype))
        self.in_names = list(in_names)
        self.out_names = out_names
        self.out_avals = out_avals
        self.zero_outs = zero_outs
        n_params = len(in_names)
        n_outs = len(out_names)
        all_in_names = in_names + out_names
        if partition_name is not None:
            all_in_names = all_in_names + [partition_name]

        def _body(*args):
            operands = list(args)
            if partition_name is not None:
                operands.append(bass2jax.partition_id_tensor())
            outs = bass2jax._bass_exec_p.bind(
                *operands,
                out_avals=tuple(out_avals),
                in_names=tuple(all_in_names),
                out_names=tuple(out_names),
                lowering_input_output_aliases=(),
                sim_require_finite=True,
                sim_require_nnan=True,
                nc=nc,
            )
            return tuple(outs)

        devices = jax.devices()[:NCORES]
        self.mesh = Mesh(np.asarray(devices), ("core",))
        self.sharding = NamedSharding(self.mesh, PartitionSpec("core"))
        in_specs = (PartitionSpec("core"),) * (n_params + n_outs)
        out_specs = (PartitionSpec("core"),) * n_outs
        self.fn = jax.jit(
            shard_map(_body, mesh=self.mesh, in_specs=in_specs,
                      out_specs=out_specs, check_rep=False),
            donate_argnums=tuple(range(n_params, n_params + n_outs)),
            keep_unused=True,
        )

    def concat_inputs(self, in_maps):
        return [
            np.concatenate([np.asarray(in_maps[c][name])
                            for c in range(NCORES)], axis=0)
            for name in self.in_names
        ]

    def device_put_inputs(self, concat_in):
        return [self.jax.device_put(a, self.sharding) for a in concat_in]

    def __call__(self, concat_in):
        zeros = [np.zeros((NCORES * z.shape[0], *z.shape[1:]), z.dtype)
                 for z in self.zero_outs]
        out_arrs = self.fn(*concat_in, *zeros)
        out_arrs = [np.asarray(a) for a in out_arrs]
        return [
            {name: out_arrs[i].reshape(NCORES, *self.out_avals[i].shape)[c]
             for i, name in enumerate(self.out_names)}
            for c in range(NCORES)
        ]


_CACHE = {}


def _get_runner(C_j, TC):
    key = (C_j, TC, str(DT_TBL))
    if key not in _CACHE:
        _CACHE[key] = _Runner(_build_program(C_j, TC))
    return _CACHE[key]


def make_in_maps(x, edge_index, W0, b0, W1, b1, pool_w, W_lin, b_lin):
    x = np.asarray(x, np.float32)
    x_pad = np.zeros((NPAD, D), np.float32)
    x_pad[:N] = x
    xT = np.ascontiguousarray(x_pad.T)

    idx_row, col_lcl, C_j, TC = _preprocess(np.asarray(edge_index))

    pw = np.asarray(pool_w, np.float32)
    pwn = (pw / np.linalg.norm(pw)).astype(np.float32)

    def bc(v, n):
        return np.ascontiguousarray(
            np.broadcast_to(np.asarray(v, np.float32), (128, n)))

    common = {
        "xT": xT,
        "W0": np.asarray(W0, np.float32),
        "W1": np.asarray(W1, np.float32),
        "Wl": np.asarray(W_lin, np.float32),
        "b0b": bc(b0, D),
        "b1b": bc(b1, D),
        "pwb": bc(pwn, D),
        "blb": bc(b_lin, OUT),
    }
    in_maps = [
        dict(common, idxs=np.ascontiguousarray(idx_row[k]),
             cols=np.ascontiguousarray(col_lcl[k]))
        for k in range(NCORES)
    ]
    return in_maps, C_j, TC


def kernel(x, edge_index, batch, W0, b0, W1, b1, pool_w, W_lin, b_lin):
    in_maps, C_j, TC = make_in_maps(x, edge_index, W0, b0, W1, b1,
                                    pool_w, W_lin, b_lin)
    runner = _get_runner(C_j, TC)
    res = runner(runner.concat_inputs(in_maps))
    out = np.concatenate([res[k]["out"] for k in range(NCORES)], axis=0)
    return np.ascontiguousarray(out[:G])
